# revision 1
# baseline (speedup 1.0000x reference)
"""GrowableAttention (GQA + RoPE + full softmax attention + o_proj) on 8 TRN2 cores.

Sharding: 8 cores = 2 batches x 4 query-blocks of 512 tokens. Each core
computes K/V for its whole batch (redundant across the 4 cores of a batch,
but removes all collectives), attends its 512 queries against all 2048
keys for all 16 heads, and produces a disjoint [512, 2048] row-slice of
the output. Host-side work is only transpose/cast/permute/concat.

Compute dtype: bf16 matmul inputs (PE runs fp32 matmul at 1/4 rate), fp32
PSUM accumulation throughout. Scores are built transposed [kt, qt] so the
softmax denominator is a ones-matmul and attn@v needs no transposes.
RoPE (half-split) is fused into the PSUM->SBUF eviction on DVE; the
1/sqrt(128) scale is folded into the q-side cos/sin tables. The key axis
is rotated per-core (np.roll) so every core's query block sits at kt
columns 0:512 -> one SPMD program, per-core behavior only via data.
"""

import math
import sys

sys.path.insert(0, "/opt/trn_rl_repo")

import ml_dtypes
import numpy as np

import concourse.bass as bass
from concourse import bacc
import concourse.mybir as mybir
from concourse.bass_utils import run_bass_kernel_spmd
from concourse.tile import TileContext

BF16 = ml_dtypes.bfloat16

NH, NKV, HD = 16, 4, 128
B, S, H = 2, 2048, 2048
T = 512           # queries per core
R = HD // 2       # rope half = 64
HT = H // 128     # 16 hidden k-tiles
KT = S // 128     # 16 key tiles
NCORES = 8
ROPE_THETA = 10000.0

_PROG = None
LAST_RESULTS = None  # BassKernelResults of the most recent run (for test.py)


def _build(upto="C"):
    nc = bacc.Bacc("TRN2", target_bir_lowering=False)
    dt = mybir.dt

    xT = nc.dram_tensor("xT", [H, S], dt.bfloat16, kind="ExternalInput")
    wqT = nc.dram_tensor("wqT", [H, NH * HD], dt.bfloat16, kind="ExternalInput")
    wkT = nc.dram_tensor("wkT", [H, NKV * HD], dt.bfloat16, kind="ExternalInput")
    wvT = nc.dram_tensor("wvT", [H, NKV * HD], dt.bfloat16, kind="ExternalInput")
    woT = nc.dram_tensor("woT", [NH * HD, H], dt.bfloat16, kind="ExternalInput")
    cosq = nc.dram_tensor("cosq", [R, T], dt.bfloat16, kind="ExternalInput")
    sinq = nc.dram_tensor("sinq", [R, T], dt.bfloat16, kind="ExternalInput")
    cosk = nc.dram_tensor("cosk", [R, S], dt.bfloat16, kind="ExternalInput")
    sink = nc.dram_tensor("sink", [R, S], dt.bfloat16, kind="ExternalInput")
    ones_b = nc.dram_tensor("ones_b", [128, 1], dt.bfloat16, kind="ExternalInput")
    ones_f = nc.dram_tensor("ones_f", [1, 128], dt.float32, kind="ExternalInput")
    out_d = nc.dram_tensor("out", [T, H], dt.float32, kind="ExternalOutput")

    Exp = mybir.ActivationFunctionType.Exp

    with TileContext(nc) as tc:
        with tc.tile_pool(name="persist", bufs=1) as pp:
            # ---- persistent SBUF ----
            xts = [[pp.tile([128, 512], dt.bfloat16, tag=f"xt{hi}_{ns}",
                            name=f"xt{hi}_{ns}") for ns in range(4)]
                   for hi in range(HT)]
            def dma_x(hi, ns):
                nc.sync.dma_start(
                    out=xts[hi][ns],
                    in_=xT[hi * 128:(hi + 1) * 128, ns * 512:(ns + 1) * 512])
            cq = pp.tile([R, T], dt.bfloat16, tag="cq", name="cq")
            nc.sync.dma_start(out=cq, in_=cosq[:, :])
            sq = pp.tile([R, T], dt.bfloat16, tag="sq", name="sq")
            nc.sync.dma_start(out=sq, in_=sinq[:, :])
            ck = pp.tile([R, S], dt.bfloat16, tag="ck", name="ck")
            nc.sync.dma_start(out=ck, in_=cosk[:, :])
            sk = pp.tile([R, S], dt.bfloat16, tag="sk", name="sk")
            nc.sync.dma_start(out=sk, in_=sink[:, :])
            onb = pp.tile([128, 1], dt.bfloat16, tag="onb", name="onb")
            nc.sync.dma_start(out=onb, in_=ones_b[:, :])
            onf = pp.tile([1, 128], dt.float32, tag="onf", name="onf")
            nc.sync.dma_start(out=onf, in_=ones_f[:, :])

            kts = [pp.tile([128, S], dt.bfloat16, tag=f"k{i}", name=f"k{i}") for i in range(NKV)]
            qts = [pp.tile([128, T], dt.bfloat16, tag=f"q{i}", name=f"q{i}") for i in range(NH)]
            vts = [pp.tile([128, NKV * HD], dt.bfloat16, tag=f"v{i}", name=f"v{i}")
                   for i in range(KT)]
            aot = [pp.tile([128, T], dt.bfloat16, tag=f"ao{i}", name=f"ao{i}") for i in range(NH)]

            # ================= phase A: projections =================
            with (
                tc.tile_pool(name="wk16", bufs=1) as wkp,
                tc.tile_pool(name="wv16", bufs=1) as wvp,
                tc.tile_pool(name="wqs", bufs=3) as wqp,
                tc.tile_pool(name="rtmp", bufs=4) as rt,
                tc.tile_pool(name="psA", bufs=4, space="PSUM") as pa,
                tc.tile_pool(name="psQ", bufs=4, space="PSUM") as paq,
            ):
                def rope_evict(ps, cos_t, sin_t, dst, col0, ncol):
                    # dst[0:64]  = ps[0:64]*cos - ps[64:128]*sin
                    # dst[64:128]= ps[64:128]*cos + ps[0:64]*sin
                    t1 = rt.tile([R, ncol], dt.float32, tag="r1", name="r1")
                    t2 = rt.tile([R, ncol], dt.float32, tag="r2", name="r2")
                    nc.vector.tensor_mul(out=t1, in0=ps[0:R, :], in1=cos_t)
                    nc.vector.tensor_mul(out=t2, in0=ps[R:128, :], in1=sin_t)
                    nc.vector.tensor_sub(
                        out=dst[0:R, col0:col0 + ncol], in0=t1, in1=t2)
                    t3 = rt.tile([R, ncol], dt.float32, tag="r1", name="r1")
                    t4 = rt.tile([R, ncol], dt.float32, tag="r2", name="r2")
                    nc.vector.tensor_mul(out=t3, in0=ps[R:128, :], in1=cos_t)
                    nc.vector.tensor_mul(out=t4, in0=ps[0:R, :], in1=sin_t)
                    nc.vector.tensor_add(
                        out=dst[R:128, col0:col0 + ncol], in0=t3, in1=t4)

                # K projection (+rope): kts[kh] = [128 d, S kt]
                wk_t = []
                for hi in range(HT):
                    w = wkp.tile([128, NKV * HD], dt.bfloat16, tag=f"wk{hi}",
                                 name=f"wk{hi}")
                    nc.sync.dma_start(out=w, in_=wkT[hi * 128:(hi + 1) * 128, :])
                    wk_t.append(w)
                    dma_x(hi, 0)
                for ns in range(1, 4):
                    for hi in range(HT):
                        dma_x(hi, ns)
                # interleave: kT column-block ns, then Q quarter ns
                for ns in range(4):
                    for kh in range(NKV):
                        ps = pa.tile([128, 512], dt.float32, tag="pa", name="pa")
                        for hi in range(HT):
                            nc.tensor.matmul(
                                out=ps,
                                lhsT=wk_t[hi][:, kh * HD:(kh + 1) * HD],
                                rhs=xts[hi][ns],
                                start=(hi == 0), stop=(hi == HT - 1))
                        rope_evict(ps, ck[:, ns * 512:(ns + 1) * 512],
                                   sk[:, ns * 512:(ns + 1) * 512],
                                   kts[kh], ns * 512, 512)
                    # Q quarter ns: heads 4*ns .. 4*ns+3
                    pq = [paq.tile([128, T], dt.float32, tag="pq", name="pq")
                          for _ in range(4)]
                    for hi in range(HT):
                        w = wqp.tile([128, 4 * HD], dt.bfloat16, tag="wq",
                                     name="wq")
                        nc.sync.dma_start(
                            out=w,
                            in_=wqT[hi * 128:(hi + 1) * 128,
                                    ns * 512:(ns + 1) * 512])
                        for h4 in range(4):
                            nc.tensor.matmul(
                                out=pq[h4],
                                lhsT=w[:, h4 * HD:(h4 + 1) * HD],
                                rhs=xts[hi][0],
                                start=(hi == 0), stop=(hi == HT - 1))
                    for h4 in range(4):
                        rope_evict(pq[h4], cq, sq, qts[ns * 4 + h4], 0, T)

                # V projection: vts[kt] = [128 kt, NKV*HD]
                wv_t = []
                for hi in range(HT):
                    w = wvp.tile([128, NKV * HD], dt.bfloat16, tag=f"wv{hi}", name=f"wv{hi}")
                    nc.sync.dma_start(out=w, in_=wvT[hi * 128:(hi + 1) * 128, :])
                    wv_t.append(w)
                for kt in range(KT):
                    ps = pa.tile([128, NKV * HD], dt.float32, tag="pa", name="pa")
                    for hi in range(HT):
                        nc.tensor.matmul(
                            out=ps,
                            lhsT=xts[hi][kt // 4][:, (kt % 4) * 128:
                                                  (kt % 4 + 1) * 128],
                            rhs=wv_t[hi],
                            start=(hi == 0), stop=(hi == HT - 1))
                    nc.scalar.copy(out=vts[kt], in_=ps)

            # ================= phase B: attention =================
            if upto in ("B", "C"):
              with (
                tc.tile_pool(name="expp", bufs=6) as ep,
                tc.tile_pool(name="smallf", bufs=4) as sf,
                tc.tile_pool(name="wos", bufs=18) as wop,
                tc.tile_pool(name="outp", bufs=3) as op_,
              ):
                with (
                    tc.tile_pool(name="psS", bufs=4, space="PSUM") as pS,
                    tc.tile_pool(name="psO", bufs=2, space="PSUM") as pO,
                    tc.tile_pool(name="psD", bufs=2, space="PSUM") as pD,
                ):
                    # prefetch all wo tiles early (DMA idle during B)
                    wo_t = []
                    for mh in range(2):
                        for h in range(NH):
                            if len(wo_t) < 16:
                                w = wop.tile([128, H // 2], dt.bfloat16,
                                             tag="wo", name="wo")
                                nc.sync.dma_start(
                                    out=w,
                                    in_=woT[h * 128:(h + 1) * 128,
                                            mh * (H // 2):(mh + 1) * (H // 2)])
                                wo_t.append((mh, h, w))
                    for h in range(NH):
                        kh = h // (NH // NKV)
                        po = pO.tile([128, T], dt.float32, tag="po", name="po")
                        pd = pD.tile([1, T], dt.float32, tag="pd", name="pd")
                        for kt in range(KT):
                            ps = pS.tile([128, T], dt.float32, tag="ps",
                                         name="ps")
                            nc.tensor.matmul(
                                out=ps,
                                lhsT=kts[kh][:, kt * 128:(kt + 1) * 128],
                                rhs=qts[h], start=True, stop=True)
                            et = ep.tile([128, T], dt.bfloat16, tag="et",
                                         name="et")
                            nc.scalar.activation(out=et, in_=ps, func=Exp)
                            nc.tensor.matmul(
                                out=po,
                                lhsT=vts[kt][:, kh * HD:(kh + 1) * HD],
                                rhs=et,
                                start=(kt == 0), stop=(kt == KT - 1))
                            nc.tensor.matmul(
                                out=pd, lhsT=onb, rhs=et,
                                start=(kt == 0), stop=(kt == KT - 1))
                        rc = sf.tile([1, T], dt.float32, tag="rc", name="rc")
                        nc.vector.reciprocal(out=rc, in_=pd)
                        bc = sf.tile([128, T], dt.float32, tag="bc", name="bc")
                        nc.gpsimd.partition_broadcast(bc, rc)
                        nc.vector.tensor_mul(out=aot[h], in0=po, in1=bc)

                # ================= phase C: o_proj =================
                if upto == "C":
                  with tc.tile_pool(name="psC", bufs=8, space="PSUM") as pC:
                    prefetched = {(mh, h): w for mh, h, w in wo_t}
                    for mh in range(2):
                        pcs = [pC.tile([128, 512], dt.float32, tag="pc",
                                       name="pc") for _ in range(8)]
                        for h in range(NH):
                            w = prefetched.get((mh, h))
                            if w is None:
                                w = wop.tile([128, H // 2], dt.bfloat16,
                                             tag="wo", name="wo")
                                nc.sync.dma_start(
                                    out=w,
                                    in_=woT[h * 128:(h + 1) * 128,
                                            mh * (H // 2):(mh + 1) * (H // 2)])
                            for tm in range(T // 128):
                                for nsi in range(2):
                                    nc.tensor.matmul(
                                        out=pcs[tm * 2 + nsi],
                                        lhsT=aot[h][:, tm * 128:(tm + 1) * 128],
                                        rhs=w[:, nsi * 512:(nsi + 1) * 512],
                                        start=(h == 0), stop=(h == NH - 1))
                        for tm in range(T // 128):
                            for nsi in range(2):
                                ot = op_.tile([128, 512], dt.float32, tag="ot",
                                              name="ot")
                                nc.vector.tensor_copy(out=ot,
                                                      in_=pcs[tm * 2 + nsi])
                                nc.sync.dma_start(
                                    out=out_d[tm * 128:(tm + 1) * 128,
                                              mh * 1024 + nsi * 512:
                                              mh * 1024 + (nsi + 1) * 512],
                                    in_=ot)
    nc.finalize()
    return nc


def _prep_inputs(hidden_states, Wq, Wk, Wv, Wo):
    inv = 1.0 / (ROPE_THETA ** (np.arange(0, HD, 2, dtype=np.float32) / HD))
    pos = np.arange(S, dtype=np.float32)
    fr = inv[:, None] * pos[None, :]            # [R, S]
    cosk = np.cos(fr).astype(np.float32)
    sink = np.sin(fr).astype(np.float32)
    sc = np.float32(1.0 / math.sqrt(HD))

    wqT = np.ascontiguousarray(np.asarray(Wq).T).astype(BF16)
    wkT = np.ascontiguousarray(np.asarray(Wk).T).astype(BF16)
    wvT = np.ascontiguousarray(np.asarray(Wv).T).astype(BF16)
    woT = np.ascontiguousarray(np.asarray(Wo).T).astype(BF16)
    ones_b = np.ones((128, 1), BF16)
    ones_f = np.ones((1, 128), np.float32)
    hs = np.asarray(hidden_states)

    in_maps = []
    for c in range(NCORES):
        b, qb = divmod(c, 4)
        perm = np.roll(np.arange(S), -qb * T)
        xTp = np.ascontiguousarray(hs[b].T[:, perm]).astype(BF16)
        in_maps.append({
            "xT": xTp,
            "wqT": wqT, "wkT": wkT, "wvT": wvT, "woT": woT,
            "cosq": (cosk[:, qb * T:(qb + 1) * T] * sc).astype(BF16),
            "sinq": (sink[:, qb * T:(qb + 1) * T] * sc).astype(BF16),
            "cosk": np.ascontiguousarray(cosk[:, perm]).astype(BF16),
            "sink": np.ascontiguousarray(sink[:, perm]).astype(BF16),
            "ones_b": ones_b, "ones_f": ones_f,
        })
    return in_maps


def kernel(hidden_states, Wq, Wk, Wv, Wo, _trace=False):
    global _PROG, LAST_RESULTS
    if _PROG is None:
        _PROG = _build()
    in_maps = _prep_inputs(hidden_states, Wq, Wk, Wv, Wo)
    res = run_bass_kernel_spmd(
        _PROG, in_maps, core_ids=list(range(NCORES)), trace=_trace)
    LAST_RESULTS = res
    full = np.empty((B, S, H), np.float32)
    for c in range(NCORES):
        b, qb = divmod(c, 4)
        full[b, qb * T:(qb + 1) * T, :] = res.results[c]["out"]
    return full

